# revision 9
# baseline (speedup 1.0000x reference)
"""Trainium2 Bass kernel for style-modulated 3D conv (DMSRStyleConv).

Math (per sample b):
  s[ci]      = style[b] @ style_w.T + style_b                  # [Cin]
  w_mod      = weight * s[None, :, None,None,None]             # [Cout,Cin,3,3,3]
  var[co]    = sum_{ci,taps} w_mod^2
  y[b]       = conv3d_valid(x[b], w_mod) * rsqrt(var+eps)[co] + bias[co]

Strategy: data-parallel over batch across 8 cores (1 sample each).
Winograd F(2,3) along the depth axis: for each output-depth PAIR
(od=2j, 2j+1), build 4 transformed input planes from slices 2j..2j+3
  p0 = s0 - s2;  p1 = s1 + s2;  p2 = s1 - s2;  p3 = s1 - s3
and 4 transformed weight sets over the kd taps
  W0 = g0;  W1 = (g0+g1+g2)/2;  W2 = -(g0-g1+g2)/2;  W3 = g2
(W2 negated to pair with the flipped p2 so all input ops are plain
add/sub).  Then m_p = conv2d_valid(p_p, W_p) over the 9 (kh,kw) taps
-- 36 accumulating matmuls per pair instead of 54 for direct conv
(1.5x less TensorE work).  Inverse transform on VectorE in fp16:
  y[2j]   = m0 + m1 + m2
  y[2j+1] = m1 - m2 - m3
The demod scale d[co] and bias fold into the PSUM->SBUF m-plane
eviction on ScalarE (activation Identity, per-partition scale+bias:
m0 gets +bias, m3 gets -bias, so the inverse sums carry bias).

Layout: PE in 64x64 array-tiling mode.  SBUF slice tiles stack output
row-half A (input rows 0:25) on partitions 0:64 and half B (rows
23:48) on partitions 64:128; psum col-group alternates per row-block
so 4 PE quadrant tiles run concurrently.  Output is written to HBM as
fp16 (halves store traffic); the host converts to fp32.
"""

import numpy as np

import concourse.bass as bass
import concourse.tile as tile
from concourse import bacc, mybir
from concourse.bass_utils import run_bass_kernel_spmd

F32 = mybir.dt.float32
F16 = mybir.dt.float16
EPS = 1e-8
N_CORES = 8
CIN = 64
COUT = 64
KK = 3
NTAP = KK * KK * KK
ID = mybir.ActivationFunctionType.Identity


def conv_body(ctx, tc, y_ap, x_ap, st4_ap, swt_ap, stb_ap, bias_ap, wt_ap,
              D, H, W, repeat=1):
    nc = tc.nc
    OD, OH, OW = D - 2, H - 2, W - 2
    assert OH % 2 == 0 and OD % 2 == 0
    hA = OH // 2          # output rows per half (23)
    SR = hA + 2           # input rows per half (halo, 25)
    NP = OD // 2          # output-depth pairs (23)
    rmax = 512 // OW
    blocks = []
    r0 = 0
    while r0 < hA:
        R = min(rmax, hA - r0)
        blocks.append((r0, R))
        r0 += R

    const_pool = ctx.enter_context(tc.tile_pool(name="const", bufs=1))
    prep_psum = ctx.enter_context(
        tc.tile_pool(name="prep_psum", bufs=1, space="PSUM"))
    conv_psum = ctx.enter_context(
        tc.tile_pool(name="conv_psum", bufs=6, space="PSUM"))
    slice_raw_pool = ctx.enter_context(tc.tile_pool(name="slraw", bufs=3))
    slice_pool = ctx.enter_context(tc.tile_pool(name="slices", bufs=6))
    plane_pool = ctx.enter_context(tc.tile_pool(name="planes", bufs=8))
    m_pool = ctx.enter_context(tc.tile_pool(name="mplanes", bufs=8))
    out_pool = ctx.enter_context(tc.tile_pool(name="outs", bufs=8))

    def body(_i=None):
        # ---------------- prep: s = style @ style_w.T + style_b --------------
        stin = const_pool.tile([128, 4], F32, tag="stin")
        for c in range(4):
            nc.sync.dma_start(stin[:, c:c + 1], st4_ap[c])
        swt_t = const_pool.tile([128, 256], F32, tag="swt")
        for c in range(4):
            nc.sync.dma_start(swt_t[:, c * 64:(c + 1) * 64], swt_ap[c])
        stb_col = const_pool.tile([128, 1], F32, tag="stb")
        nc.sync.dma_start(stb_col[0:64, :], stb_ap[:])
        nc.sync.dma_start(stb_col[64:128, :], stb_ap[:])
        bias_col = const_pool.tile([128, 1], F32, tag="bias")
        nc.sync.dma_start(bias_col[0:64, :], bias_ap[:])
        nc.sync.dma_start(bias_col[64:128, :], bias_ap[:])
        w_raw = const_pool.tile([128, NTAP * COUT], F32, tag="wraw")
        nc.sync.dma_start(w_raw[0:64, :], wt_ap[:])
        nc.sync.dma_start(w_raw[64:128, :], wt_ap[:])
        ones_t = const_pool.tile([128, 1], F32, tag="ones")
        nc.vector.memset(ones_t[:], 1.0)
        eps_t = const_pool.tile([128, 1], F32, tag="eps")
        nc.vector.memset(eps_t[:], EPS)
        nbias_col = const_pool.tile([128, 1], F32, tag="nbias")
        nc.vector.tensor_scalar_mul(nbias_col[:], bias_col[:], -1.0)

        psum_s = prep_psum.tile([128, 1], F32, tag="prep")
        for half in (0, 64):
            for c in range(4):
                nc.tensor.matmul(
                    psum_s[half:half + 64, :],
                    lhsT=swt_t[:, c * 64:(c + 1) * 64],
                    rhs=stin[:, c:c + 1],
                    start=(c == 0), stop=(c == 3))
        s_col = const_pool.tile([128, 1], F32, tag="scol")
        nc.vector.tensor_add(s_col[:], psum_s[:], stb_col[:])

        # modulated weights, both partition halves (fp16: rounded on write)
        w2 = const_pool.tile([128, NTAP * COUT], F16, tag="w2")
        nc.vector.tensor_scalar_mul(w2[:], w_raw[:], s_col[:])

        # demod: var[co] = sum w2^2 over (ci, taps); use lower half only
        sq = const_pool.tile([128, NTAP * COUT], F32, tag="sq")
        nc.vector.tensor_mul(sq[0:64, :], w2[0:64, :], w2[0:64, :])
        psum_var = prep_psum.tile([128, 64], F32, tag="prep")
        for t in range(NTAP):
            nc.tensor.matmul(
                psum_var[0:1, :],
                lhsT=ones_t[0:64, :],
                rhs=sq[0:64, t * 64:(t + 1) * 64],
                start=(t == 0), stop=(t == NTAP - 1))
        std_t = const_pool.tile([128, 64], F32, tag="std")
        nc.scalar.activation(std_t[0:1, :], psum_var[0:1, :],
                             mybir.ActivationFunctionType.Sqrt,
                             bias=eps_t[0:1, :])
        dinv = const_pool.tile([128, 64], F32, tag="dinv")
        nc.vector.reciprocal(dinv[0:1, :], std_t[0:1, :])
        # transpose [1,64] -> [64,1] on both psum halves via K=1 matmul
        psum_d = prep_psum.tile([128, 1], F32, tag="prep")
        for half in (0, 64):
            nc.tensor.matmul(
                psum_d[half:half + 64, :],
                lhsT=dinv[0:1, :],
                rhs=ones_t[0:1, :],
                start=True, stop=True)
        d_col = const_pool.tile([128, 1], F32, tag="dcol")
        nc.vector.tensor_copy(d_col[:], psum_d[:])

        # --------- Winograd weight transform along kd (fp16) -----------------
        # w2 layout: [ci, kd, khkw, co] with free = kd*576 + khkw*64 + co
        NF = 9 * COUT   # 576
        wv = w2[:, :].rearrange("p (kd f) -> p kd f", kd=3)
        wg = [const_pool.tile([128, NF], F16, tag=f"wg{p}", name=f"wg{p}")
              for p in range(4)]
        wu = const_pool.tile([128, NF], F16, tag="wu")
        nc.vector.tensor_copy(wg[0][:], wv[:, 0, :])
        nc.vector.tensor_copy(wg[3][:], wv[:, 2, :])
        nc.vector.tensor_add(wu[:], wv[:, 0, :], wv[:, 2, :])     # g0+g2
        nc.vector.tensor_add(wg[1][:], wu[:], wv[:, 1, :])        # g0+g1+g2
        nc.vector.tensor_scalar_mul(wg[1][:], wg[1][:], 0.5)
        nc.vector.tensor_sub(wg[2][:], wv[:, 1, :], wu[:])        # g1-g0-g2
        nc.vector.tensor_scalar_mul(wg[2][:], wg[2][:], 0.5)      # = -W2

        # ---------------- conv ----------------------------------------------
        def load_slice(d):
            raw = slice_raw_pool.tile([128, SR, W], F32, tag="slr", name="slr")
            nc.sync.dma_start(raw[0:64, :, :], x_ap[:, d, 0:SR, :])
            nc.sync.dma_start(raw[64:128, :, :], x_ap[:, d, hA:hA + SR, :])
            t = slice_pool.tile([128, SR, W], F16, tag="sl", name="sl")
            nc.vector.tensor_copy(t[:, :, :], raw[:, :, :])
            return t

        def tf_planes(j):
            s0, s1, s2, s3 = (sl[2 * j], sl[2 * j + 1],
                              sl[2 * j + 2], sl[2 * j + 3])
            pls = [plane_pool.tile([128, SR, W], F16, tag="pl", name="pl")
                   for _ in range(4)]
            nc.vector.tensor_sub(pls[0][:], s0[:], s2[:])
            nc.vector.tensor_add(pls[1][:], s1[:], s2[:])
            nc.vector.tensor_sub(pls[2][:], s1[:], s2[:])
            nc.vector.tensor_sub(pls[3][:], s1[:], s3[:])
            return pls

        # software pipeline: input transforms run one pair ahead so the
        # DVE FIFO never blocks next pair's planes behind this pair's
        # inverse (which stalls on this pair's matmuls+evictions).
        sl = {d: load_slice(d) for d in range(4)}
        pl_ring = {0: tf_planes(0)}
        del sl[0], sl[1]
        for j in range(NP):
            if j + 1 < NP:
                sl[2 * j + 4] = load_slice(2 * j + 4)
                sl[2 * j + 5] = load_slice(2 * j + 5)
                pl_ring[j + 1] = tf_planes(j + 1)
                del sl[2 * j + 2], sl[2 * j + 3]
            planes = pl_ring.pop(j)

            ms = [m_pool.tile([128, hA, OW], F16, tag="m", name="m")
                  for _ in range(4)]
            for p in range(4):
                psums = [
                    conv_psum.tile([128, 512], F32, tag="cps", name="cps")
                    [:, 0:R * OW].rearrange("p (r w) -> p r w", w=OW)
                    for (_r0, R) in blocks]
                pl = planes[p]
                for t in range(9):
                    kh, kw = divmod(t, 3)
                    wlo = wg[p][0:64, t * 64:(t + 1) * 64]
                    whi = wg[p][64:128, t * 64:(t + 1) * 64]
                    first = (t == 0)
                    last = (t == 8)
                    for bi, (r0, R) in enumerate(blocks):
                        ps = psums[bi]
                        rhsA = pl[0:64, r0 + kh:r0 + kh + R, kw:kw + OW]
                        rhsB = pl[64:128, r0 + kh:r0 + kh + R, kw:kw + OW]
                        if bi % 2 == 0:   # A -> lower psum half, B -> upper
                            nc.tensor.matmul(ps[0:64, :, :], lhsT=wlo,
                                             rhs=rhsA, start=first, stop=last,
                                             skip_group_check=True)
                            nc.tensor.matmul(ps[64:128, :, :], lhsT=whi,
                                             rhs=rhsB, start=first, stop=last,
                                             skip_group_check=True)
                        else:             # A -> upper, B -> lower
                            nc.tensor.matmul(ps[64:128, :, :], lhsT=wlo,
                                             rhs=rhsA, start=first, stop=last,
                                             skip_group_check=True)
                            nc.tensor.matmul(ps[0:64, :, :], lhsT=whi,
                                             rhs=rhsB, start=first, stop=last,
                                             skip_group_check=True)
                # evict m-plane: m = psum * d[co] (+/- bias for p0/p3);
                # un-swap odd blocks so m layout is A on 0:64, B on 64:128
                bias_arg = (bias_col if p == 0 else
                            nbias_col if p == 3 else None)
                for bi, (r0, R) in enumerate(blocks):
                    ps = psums[bi]
                    mdst = ms[p][:, r0:r0 + R, :]
                    if bi % 2 == 0:
                        if bias_arg is None:
                            nc.scalar.activation(mdst, ps[:, :, :], ID,
                                                 scale=d_col[:])
                        else:
                            nc.scalar.activation(mdst, ps[:, :, :], ID,
                                                 scale=d_col[:],
                                                 bias=bias_arg[:])
                    else:
                        dA = ms[p][0:64, r0:r0 + R, :]
                        dB = ms[p][64:128, r0:r0 + R, :]
                        if bias_arg is None:
                            nc.scalar.activation(dA, ps[64:128, :, :], ID,
                                                 scale=d_col[0:64, :])
                            nc.scalar.activation(dB, ps[0:64, :, :], ID,
                                                 scale=d_col[64:128, :])
                        else:
                            nc.scalar.activation(dA, ps[64:128, :, :], ID,
                                                 scale=d_col[0:64, :],
                                                 bias=bias_arg[0:64, :])
                            nc.scalar.activation(dB, ps[0:64, :, :], ID,
                                                 scale=d_col[64:128, :],
                                                 bias=bias_arg[64:128, :])

            # inverse transform (fp16, 2x mode): y_e = m0+m1+m2 (+bias via m0)
            #                                    y_o = m1-m2-m3 (+bias via m3)
            t_e = out_pool.tile([128, hA, OW], F16, tag="te", name="te")
            t_o = out_pool.tile([128, hA, OW], F16, tag="to", name="to")
            y_e = out_pool.tile([128, hA, OW], F16, tag="ye", name="ye")
            y_o = out_pool.tile([128, hA, OW], F16, tag="yo", name="yo")
            nc.vector.tensor_add(t_e[:], ms[0][:], ms[1][:])
            nc.vector.tensor_add(y_e[:], t_e[:], ms[2][:])
            nc.vector.tensor_sub(t_o[:], ms[1][:], ms[2][:])
            nc.vector.tensor_sub(y_o[:], t_o[:], ms[3][:])

            nc.sync.dma_start(y_ap[:, 2 * j, 0:hA, :], y_e[0:64, :, :])
            nc.sync.dma_start(y_ap[:, 2 * j, hA:OH, :], y_e[64:128, :, :])
            nc.sync.dma_start(y_ap[:, 2 * j + 1, 0:hA, :], y_o[0:64, :, :])
            nc.sync.dma_start(y_ap[:, 2 * j + 1, hA:OH, :], y_o[64:128, :, :])

    if repeat == 1:
        body()
    else:
        with tc.For_i(0, repeat, 1) as i:
            body(i)


def build_bass(D=48, H=48, W=48, repeat=1, n_cores=N_CORES):
    from contextlib import ExitStack
    nc = bacc.Bacc("TRN2", target_bir_lowering=False, debug=False,
                   num_devices=n_cores)
    OD, OH, OW = D - 2, H - 2, W - 2
    x_ap = nc.dram_tensor("x", [CIN, D, H, W], F32, kind="ExternalInput").ap()
    st4_ap = nc.dram_tensor("st4", [4, 128, 1], F32, kind="ExternalInput").ap()
    swt_ap = nc.dram_tensor("swt", [4, 128, 64], F32, kind="ExternalInput").ap()
    stb_ap = nc.dram_tensor("stb", [64, 1], F32, kind="ExternalInput").ap()
    bias_ap = nc.dram_tensor("bias", [64, 1], F32, kind="ExternalInput").ap()
    wt_ap = nc.dram_tensor("wt", [64, NTAP * COUT], F32,
                           kind="ExternalInput").ap()
    y_ap = nc.dram_tensor("y", [COUT, OD, OH, OW], F16,
                          kind="ExternalOutput").ap()
    with tile.TileContext(nc) as tc:
        with ExitStack() as ctx:
            conv_body(ctx, tc, y_ap, x_ap, st4_ap, swt_ap, stb_ap, bias_ap,
                      wt_ap, D, H, W, repeat=repeat)
    nc.compile()
    return nc


def make_in_maps(x, style, weight, bias, style_w, style_b):
    B = x.shape[0]
    swt = np.ascontiguousarray(
        style_w.T.reshape(4, 128, 64).astype(np.float32))
    wt = np.ascontiguousarray(
        np.transpose(weight, (1, 2, 3, 4, 0)).reshape(CIN, NTAP * COUT)
        .astype(np.float32))
    stb = np.ascontiguousarray(style_b.reshape(64, 1).astype(np.float32))
    bi = np.ascontiguousarray(bias.reshape(64, 1).astype(np.float32))
    return [{
        "x": np.ascontiguousarray(x[b].astype(np.float32)),
        "st4": np.ascontiguousarray(style[b].reshape(4, 128, 1)
                                    .astype(np.float32)),
        "swt": swt, "stb": stb, "bias": bi, "wt": wt,
    } for b in range(B)]


_NC_CACHE = {}


def _get_nc(repeat=1):
    key = repeat
    if key not in _NC_CACHE:
        _NC_CACHE[key] = build_bass(48, 48, 48, repeat=repeat)
    return _NC_CACHE[key]


def kernel(x, style, weight, bias, style_w, style_b):
    assert x.shape == (8, CIN, 48, 48, 48), x.shape
    nc = _get_nc(1)
    in_maps = make_in_maps(x, style, weight, bias, style_w, style_b)
    res = run_bass_kernel_spmd(nc, in_maps, list(range(N_CORES)))
    y = np.stack([res.results[b]["y"] for b in range(len(in_maps))], axis=0)
    return y.astype(np.float32)
